# revision 5
# baseline (speedup 1.0000x reference)
"""LocalRNN Trainium2 kernel: GLU -> pointwise conv -> 9-step windowed LSTM.

Full inputs in, full output out. Sharding: batch across 8 cores (2 batches/core).
All matmuls in float32r (full-rate fp32). Everything on-chip lives in transposed
[feature, token] layout so the LSTM recurrence needs no transposes.
"""
from contextlib import ExitStack

import numpy as np

import concourse.bass as bass
import concourse.mybir as mybir
import concourse.tile as tile
from concourse import bacc, bass_utils
from concourse.masks import make_identity

F32 = mybir.dt.float32
F32R = mybir.dt.float32r
AF = mybir.ActivationFunctionType

N_CORES = 8
B_PER_CORE = 2          # batches per core
L = 512                 # sequence length
NT = B_PER_CORE * L     # tokens per core = 1024
D = 512                 # model dim
DH = 256                # GLU half dim
G4 = 4 * D              # 2048 gate rows
K = 9                   # window size
PAD = K - 1             # 8
LW = PAD + L            # 520: per-batch padded G row width

_cache = {}


def _build(trace=False):
    nc = bacc.Bacc(
        trn_type="TRN2", target_bir_lowering=False, debug=False, num_devices=N_CORES
    )

    x_d = nc.dram_tensor("x", [NT, D], F32, kind="ExternalInput").ap()
    convw_d = nc.dram_tensor("convw", [DH, D], F32, kind="ExternalInput").ap()   # conv_w.T
    convb_d = nc.dram_tensor("convb", [128, 4], F32, kind="ExternalInput").ap()  # conv_b [512]->[4,128].T
    wih_d = nc.dram_tensor("wih", [D, G4], F32, kind="ExternalInput").ap()       # w_ih.T
    whh_d = nc.dram_tensor("whh", [D, G4], F32, kind="ExternalInput").ap()       # w_hh.T
    bias_d = nc.dram_tensor("bias", [128, 16], F32, kind="ExternalInput").ap()   # (b_ih+b_hh) [2048]->[16,128].T
    out_d = nc.dram_tensor("out", [NT, D], F32, kind="ExternalOutput").ap()

    with tile.TileContext(nc) as tc, ExitStack() as top:
        const_pool = top.enter_context(tc.tile_pool(name="const", bufs=1))
        w_pool = top.enter_context(tc.tile_pool(name="weights", bufs=1))
        state_pool = top.enter_context(tc.tile_pool(name="state", bufs=1))

        # constants
        ident_f32 = const_pool.tile([128, 128], F32, tag="idf")
        make_identity(nc, ident_f32[:])
        ident = const_pool.tile([128, 128], F32R, tag="idr")
        nc.scalar.copy(ident[:], ident_f32[:])
        zeros8 = const_pool.tile([128, PAD], F32, tag="z8")
        nc.gpsimd.memset(zeros8[:], 0.0)
        bias_sb = const_pool.tile([128, 16], F32, tag="bias")
        nc.sync.dma_start(bias_sb[:], bias_d)
        convb_sb = const_pool.tile([128, 4], F32, tag="convb")
        nc.sync.dma_start(convb_sb[:], convb_d)

        # recurrent weights, persistent: 4 d-chunk tiles [128, 2048] f32r
        whh = []
        for dk in range(4):
            t = w_pool.tile([128, G4], F32R, tag=f"whh{dk}")
            nc.sync.dma_start(t[:], whh_d[dk * 128:(dk + 1) * 128, :].bitcast(F32R))
            whh.append(t)

        # persistent state: padded input-gate table, h (ping-pong), c
        gt = [state_pool.tile([128, B_PER_CORE * LW], F32R, tag=f"gt{i}", name=f"gt{i}")
              for i in range(16)]
        hT = [[state_pool.tile([128, NT], F32R, tag=f"h{p}_{j}", name=f"h{p}_{j}")
               for j in range(4)] for p in range(2)]
        cT = [state_pool.tile([128, NT], F32, tag=f"c{j}", name=f"c{j}") for j in range(4)]

        # ---------------- prep: GLU -> conv -> G table ----------------
        with ExitStack() as prep:
            ytp = prep.enter_context(tc.tile_pool(name="ytp", bufs=1))
            yT = [ytp.tile([128, NT], F32R, tag=f"yT{dm}", name=f"yT{dm}") for dm in range(4)]

            with ExitStack() as prepA:
                xp = prepA.enter_context(tc.tile_pool(name="xp", bufs=3))
                up = prepA.enter_context(tc.tile_pool(name="up", bufs=3))
                utp = prepA.enter_context(tc.tile_pool(name="utp", bufs=1))
                cwp = prepA.enter_context(tc.tile_pool(name="cwp", bufs=1))
                pps = prepA.enter_context(tc.tile_pool(name="pps", bufs=4, space="PSUM"))

                convw = []
                for ci in range(2):
                    t = cwp.tile([128, D], F32R, tag=f"cw{ci}")
                    nc.sync.dma_start(t[:], convw_d[ci * 128:(ci + 1) * 128, :].bitcast(F32R))
                    convw.append(t)

                # GLU + transpose u -> uT [2][128, 1024] f32r
                uT = [utp.tile([128, NT], F32R, tag=f"uT{ci}", name=f"uT{ci}")
                      for ci in range(2)]
                for ti in range(8):
                    xt = xp.tile([128, D], F32, tag="x")
                    nc.sync.dma_start(xt[:], x_d[ti * 128:(ti + 1) * 128, :])
                    sg = up.tile([128, DH], F32, tag="sg")
                    nc.scalar.activation(sg[:], xt[:, DH:D], AF.Sigmoid)
                    ut = up.tile([128, DH], F32R, tag="u")
                    nc.vector.tensor_mul(ut[:], xt[:, 0:DH], sg[:])
                    for ci in range(2):
                        ptr = pps.tile([128, 128], F32R, tag="tr")
                        nc.tensor.transpose(ptr[:], ut[:, ci * 128:(ci + 1) * 128], ident[:])
                        nc.scalar.copy(uT[ci][:, ti * 128:(ti + 1) * 128], ptr[:])

                # conv: yT[dm][128, 1024] f32r = conv_wT.T @ uT + conv_b
                for dm in range(4):
                    for half in range(2):
                        pmm = pps.tile([128, 512], F32, tag="mm")
                        for ci in range(2):
                            nc.tensor.matmul(
                                pmm[:], convw[ci][:, dm * 128:(dm + 1) * 128],
                                uT[ci][:, half * 512:(half + 1) * 512],
                                start=(ci == 0), stop=(ci == 1),
                            )
                        nc.scalar.activation(
                            yT[dm][:, half * 512:(half + 1) * 512], pmm[:],
                            AF.Identity, bias=convb_sb[:, dm:dm + 1],
                        )

            # G table: gt[i] = w_ihT.T @ yT + (b_ih+b_hh), with 8 bias-only pad
            # cols per batch. Layout per batch b: cols [b*520, b*520+8) pad,
            # [b*520+8, b*520+520) = G[b, 0:512].
            # w_ih streamed in two gd-halves to halve its SBUF footprint.
            with ExitStack() as prepB:
                wihp = prepB.enter_context(tc.tile_pool(name="wihp", bufs=1))
                ppsb = prepB.enter_context(tc.tile_pool(name="ppsb", bufs=4, space="PSUM"))
                wih = [wihp.tile([128, G4 // 2], F32R, tag=f"wih{dk}", name=f"wih{dk}")
                       for dk in range(4)]
                for hg in range(2):
                    for dk in range(4):
                        nc.sync.dma_start(
                            wih[dk][:],
                            wih_d[dk * 128:(dk + 1) * 128,
                                  hg * (G4 // 2):(hg + 1) * (G4 // 2)].bitcast(F32R),
                        )
                    for ii in range(8):
                        i = hg * 8 + ii
                        for b in range(B_PER_CORE):
                            nc.scalar.activation(
                                gt[i][:, b * LW:b * LW + PAD], zeros8[:],
                                AF.Identity, bias=bias_sb[:, i:i + 1],
                            )
                            pmm = ppsb.tile([128, 512], F32, tag="mm")
                            for dk in range(4):
                                nc.tensor.matmul(
                                    pmm[:], wih[dk][:, ii * 128:(ii + 1) * 128],
                                    yT[dk][:, b * 512:(b + 1) * 512],
                                    start=(dk == 0), stop=(dk == 3),
                                )
                            nc.scalar.activation(
                                gt[i][:, b * LW + PAD:b * LW + LW], pmm[:],
                                AF.Identity, bias=bias_sb[:, i:i + 1],
                            )

        # ---------------- LSTM steps ----------------
        with ExitStack() as run:
            psg = run.enter_context(tc.tile_pool(name="psg", bufs=2, space="PSUM"))
            tp = run.enter_context(tc.tile_pool(name="tmp", bufs=2))

            def cell(j, b, gates_in, k):
                """gates_in: list of 4 APs [128,512] (pre-activation I,F,G,O)."""
                cs = cT[j][:, b * 512:(b + 1) * 512]
                hs = hT[k % 2][j][:, b * 512:(b + 1) * 512]
                tI = tp.tile([128, 512], F32, tag="tI")
                nc.scalar.activation(tI[:], gates_in[0], AF.Sigmoid)
                tG = tp.tile([128, 512], F32, tag="tG")
                nc.scalar.activation(tG[:], gates_in[2], AF.Tanh)
                tO = tp.tile([128, 512], F32, tag="tO")
                nc.scalar.activation(tO[:], gates_in[3], AF.Sigmoid)
                t1 = tp.tile([128, 512], F32, tag="t1")
                nc.vector.tensor_mul(t1[:], tI[:], tG[:])
                if k == 0:
                    nc.vector.tensor_copy(cs, t1[:])
                else:
                    tF = tp.tile([128, 512], F32, tag="tF")
                    nc.scalar.activation(tF[:], gates_in[1], AF.Sigmoid)
                    t2 = tp.tile([128, 512], F32, tag="t2")
                    nc.vector.tensor_mul(t2[:], tF[:], cs)
                    nc.vector.tensor_add(cs, t1[:], t2[:])
                tTc = tp.tile([128, 512], F32, tag="tTc")
                nc.scalar.activation(tTc[:], cs, AF.Tanh)
                nc.vector.tensor_mul(hs, tO[:], tTc[:])

            # step 0: gates = G slice directly (h=0, c=0)
            for b in range(B_PER_CORE):
                for j in range(4):
                    gin = [gt[4 * g + j][:, b * LW:b * LW + 512] for g in range(4)]
                    cell(j, b, gin, 0)

            # steps 1..8: gates = W_hh @ h + G slice (via identity matmul)
            for k in range(1, K):
                for b in range(B_PER_CORE):
                    for j in range(4):
                        ps = []
                        for g in range(4):
                            p = psg.tile([128, 512], F32, tag=f"p{g}")
                            nc.tensor.matmul(
                                p[:], ident[:], gt[4 * g + j][:, b * LW + k:b * LW + k + 512],
                                start=True, stop=False,
                            )
                            for dk in range(4):
                                nc.tensor.matmul(
                                    p[:], whh[dk][:, (4 * g + j) * 128:(4 * g + j + 1) * 128],
                                    hT[(k + 1) % 2][dk][:, b * 512:(b + 1) * 512],
                                    start=False, stop=(dk == 3),
                                )
                            ps.append(p)
                        cell(j, b, [p[:] for p in ps], k)

        # ---------------- output: transpose h back to [token, d] ----------------
        hf = hT[(K - 1) % 2]
        with ExitStack() as fin:
            pso = fin.enter_context(tc.tile_pool(name="pso", bufs=4, space="PSUM"))
            osb = fin.enter_context(tc.tile_pool(name="osb", bufs=2))
            for ti in range(8):
                ot = osb.tile([128, D], F32, tag="o")
                for j in range(4):
                    ptr = pso.tile([128, 128], F32R, tag="tr")
                    nc.tensor.transpose(ptr[:], hf[j][:, ti * 128:(ti + 1) * 128], ident[:])
                    nc.scalar.copy(ot[:, j * 128:(j + 1) * 128], ptr[:])
                nc.sync.dma_start(out_d[ti * 128:(ti + 1) * 128, :], ot[:])

    nc.compile()
    return nc


def _make_in_maps(inputs):
    x = np.asarray(inputs["x"], dtype=np.float32)
    conv_w = np.asarray(inputs["conv_w"], dtype=np.float32)
    conv_b = np.asarray(inputs["conv_b"], dtype=np.float32)
    w_ih = np.asarray(inputs["w_ih"], dtype=np.float32)
    w_hh = np.asarray(inputs["w_hh"], dtype=np.float32)
    b_ih = np.asarray(inputs["b_ih"], dtype=np.float32)
    b_hh = np.asarray(inputs["b_hh"], dtype=np.float32)

    shared = {
        "convw": np.ascontiguousarray(conv_w.T),                      # [256, 512]
        "convb": np.ascontiguousarray(conv_b.reshape(4, 128).T),      # [128, 4]
        "wih": np.ascontiguousarray(w_ih.T),                          # [512, 2048]
        "whh": np.ascontiguousarray(w_hh.T),                          # [512, 2048]
        "bias": np.ascontiguousarray((b_ih + b_hh).reshape(16, 128).T),  # [128, 16]
    }
    in_maps = []
    for c in range(N_CORES):
        m = dict(shared)
        m["x"] = np.ascontiguousarray(
            x[c * B_PER_CORE:(c + 1) * B_PER_CORE].reshape(NT, D)
        )
        in_maps.append(m)
    return in_maps


def kernel(x, conv_w, conv_b, w_ih, w_hh, b_ih, b_hh):
    if "nc" not in _cache:
        _cache["nc"] = _build()
    nc = _cache["nc"]

    in_maps = _make_in_maps(dict(
        x=x, conv_w=conv_w, conv_b=conv_b, w_ih=w_ih, w_hh=w_hh,
        b_ih=b_ih, b_hh=b_hh,
    ))

    res = bass_utils.run_bass_kernel_spmd(nc, in_maps, core_ids=list(range(N_CORES)))
    out = np.concatenate(
        [r["out"].reshape(B_PER_CORE, L, D) for r in res.results], axis=0
    )
    return out


# revision 9
# speedup vs baseline: 1.0844x; 1.0844x over previous
"""LocalRNN Trainium2 kernel: GLU -> pointwise conv -> 9-step windowed LSTM.

Full inputs in, full output out. Sharding: batch across 8 cores (2 batches/core).

v2 design notes:
- All matmuls float32r (sustains ~1 cycle/row at N=512).
- Everything on-chip in transposed [feature, token] layout; the recurrence
  needs no transposes (h is produced by DVE directly in matmul-rhs layout).
- Conv (kernel_size=1) folded into W_ih on the host: G = (W_ih@conv_w) @ u
  with bias b_ih+b_hh+W_ih@conv_b.
- Gate rows permuted host-side to I,F,O,G so one sigmoid covers 1536
  contiguous PSUM columns.
- Input-side gates G computed once per token (9x reuse across overlapping
  windows); per-step G slice enters PSUM via an identity matmul in the same
  accumulation group as the W_hh matmuls.
- Output returned in transposed layout; host does the final transpose.
"""
from contextlib import ExitStack

import numpy as np

import concourse.bass as bass
import concourse.mybir as mybir
import concourse.tile as tile
from concourse import bacc, bass_utils
from concourse.masks import make_identity

F32 = mybir.dt.float32
F32R = mybir.dt.float32r
AF = mybir.ActivationFunctionType

N_CORES = 8
B_PER_CORE = 2          # batches per core
L = 512                 # sequence length
NT = B_PER_CORE * L     # tokens per core = 1024
D = 512                 # model dim
DH = 256                # GLU half dim
G4 = 4 * D              # 2048 gate rows
K = 9                   # window size
PAD = K - 1             # 8
LW = PAD + L            # 520: per-batch padded G row width

_cache = {}


def _build():
    nc = bacc.Bacc(
        trn_type="TRN2", target_bir_lowering=False, debug=False, num_devices=N_CORES
    )

    x_d = nc.dram_tensor("x", [NT, D], F32, kind="ExternalInput").ap()
    wf_d = nc.dram_tensor("wf", [DH, G4], F32, kind="ExternalInput").ap()    # (w_ih@conv_w).T permuted
    whh_d = nc.dram_tensor("whh", [D, G4], F32, kind="ExternalInput").ap()   # w_hh.T permuted
    bias_d = nc.dram_tensor("bias", [128, 32], F32, kind="ExternalInput").ap()
    out_d = nc.dram_tensor("out", [D, NT], F32, kind="ExternalOutput").ap()  # transposed out

    with tile.TileContext(nc) as tc, ExitStack() as top:
        const_pool = top.enter_context(tc.tile_pool(name="const", bufs=1))
        w_pool = top.enter_context(tc.tile_pool(name="weights", bufs=1))
        state_pool = top.enter_context(tc.tile_pool(name="state", bufs=1))

        ident_f32 = const_pool.tile([128, 128], F32, tag="idf")
        make_identity(nc, ident_f32[:])
        ident = const_pool.tile([128, 128], F32R, tag="idr")
        nc.scalar.copy(ident[:], ident_f32[:])
        zeros8 = const_pool.tile([128, PAD], F32, tag="z8")
        nc.gpsimd.memset(zeros8[:], 0.0)
        bias_sb = const_pool.tile([128, 32], F32, tag="bias")
        nc.sync.dma_start(bias_sb[:], bias_d)

        whh = []
        for dk in range(4):
            t = w_pool.tile([128, G4], F32R, tag=f"whh{dk}", name=f"whh{dk}")
            nc.sync.dma_start(t[:], whh_d[dk * 128:(dk + 1) * 128, :].bitcast(F32R))
            whh.append(t)

        gt = [state_pool.tile([128, B_PER_CORE * LW], F32R, tag=f"gt{i}", name=f"gt{i}")
              for i in range(16)]
        hT = [[state_pool.tile([128, NT], F32R, tag=f"h{p}_{j}", name=f"h{p}_{j}")
               for j in range(4)] for p in range(2)]
        cT = [state_pool.tile([128, NT], F32, tag=f"c{j}", name=f"c{j}") for j in range(4)]

        tp = top.enter_context(tc.tile_pool(name="tmp", bufs=2))

        def cell0(j, b):
            """step 0: c = sig(I)*tanh(G); h = sig(O)*tanh(c)."""
            cs = cT[j][:, b * 512:(b + 1) * 512]
            hs = hT[0][j][:, b * 512:(b + 1) * 512]
            g0 = b * LW  # step-0 slice offset (pad col 0..7 + G cols 0..503)
            tI = tp.tile([128, 512], F32, tag="tG", name="tI0")
            nc.scalar.activation(tI[:], gt[0 * 4 + j][:, g0:g0 + 512], AF.Sigmoid)
            tG = tp.tile([128, 512], F32, tag="tG", name="tG0")
            nc.scalar.activation(tG[:], gt[3 * 4 + j][:, g0:g0 + 512], AF.Tanh)
            tO = tp.tile([128, 512], F32, tag="tG", name="tO0")
            nc.scalar.activation(tO[:], gt[2 * 4 + j][:, g0:g0 + 512], AF.Sigmoid)
            nc.vector.tensor_mul(cs, tI[:], tG[:])
            tTc = tp.tile([128, 512], F32, tag="tTc", name="tTc0")
            nc.scalar.activation(tTc[:], cs, AF.Tanh)
            nc.vector.tensor_mul(hs, tO[:], tTc[:])

        def cell(j, b, P, k):
            """steps 1..8: full LSTM cell from psum P [128, 2048] = I|F|O|G."""
            cs = cT[j][:, b * 512:(b + 1) * 512]
            hs = hT[k % 2][j][:, b * 512:(b + 1) * 512]
            tSig = tp.tile([128, 1536], F32, tag="tSig", name="tSig")
            nc.scalar.activation(tSig[:], P[:, 0:1536], AF.Sigmoid)
            tG = tp.tile([128, 512], F32, tag="tG", name="tG")
            nc.scalar.activation(tG[:], P[:, 1536:2048], AF.Tanh)
            t1 = tp.tile([128, 512], F32, tag="t1", name="t1")
            nc.vector.tensor_mul(t1[:], tSig[:, 0:512], tG[:])
            t2 = tp.tile([128, 512], F32, tag="tG", name="t2")
            nc.vector.tensor_mul(t2[:], tSig[:, 512:1024], cs)
            nc.vector.tensor_add(cs, t1[:], t2[:])
            tTc = tp.tile([128, 512], F32, tag="tTc", name="tTc")
            nc.scalar.activation(tTc[:], cs, AF.Tanh)
            nc.vector.tensor_mul(hs, tSig[:, 1024:1536], tTc[:])
            if k == K - 1:
                nc.sync.dma_start(
                    out_d[j * 128:(j + 1) * 128, b * 512:(b + 1) * 512].bitcast(F32R),
                    hs,
                )

        # ---------------- prep: GLU -> u -> G table; step 0 interleaved ----------------
        with ExitStack() as prep:
            xp = prep.enter_context(tc.tile_pool(name="xp", bufs=2))
            up = prep.enter_context(tc.tile_pool(name="up", bufs=2))
            utp = prep.enter_context(tc.tile_pool(name="utp", bufs=1))
            wfp = prep.enter_context(tc.tile_pool(name="wfp", bufs=1))
            pps = prep.enter_context(tc.tile_pool(name="pps", bufs=4, space="PSUM"))

            wf = []
            for ck in range(2):
                t = wfp.tile([128, G4], F32R, tag=f"wf{ck}", name=f"wf{ck}")
                nc.sync.dma_start(t[:], wf_d[ck * 128:(ck + 1) * 128, :].bitcast(F32R))
                wf.append(t)

            uT = [utp.tile([128, NT], F32R, tag=f"uT{ci}", name=f"uT{ci}")
                  for ci in range(2)]
            for ti in range(8):
                xt = xp.tile([128, D], F32, tag="x", name="xt")
                nc.sync.dma_start(xt[:], x_d[ti * 128:(ti + 1) * 128, :])
                sg = up.tile([128, DH], F32, tag="sg", name="sg")
                nc.scalar.activation(sg[:], xt[:, DH:D], AF.Sigmoid)
                ut = up.tile([128, DH], F32R, tag="u", name="ut")
                nc.vector.tensor_mul(ut[:], xt[:, 0:DH], sg[:])
                for ci in range(2):
                    ptr = pps.tile([128, 128], F32R, tag="tr", name="ptr")
                    nc.tensor.transpose(ptr[:], ut[:, ci * 128:(ci + 1) * 128], ident[:])
                    nc.vector.tensor_copy(uT[ci][:, ti * 128:(ti + 1) * 128], ptr[:])

            # G table per batch, then immediately step-0 cells for that batch
            for b in range(B_PER_CORE):
                for i in range(16):
                    nc.scalar.activation(
                        gt[i][:, b * LW:b * LW + PAD], zeros8[:],
                        AF.Identity, bias=bias_sb[:, 16 + i:16 + i + 1],
                    )
                    pmm = pps.tile([128, 512], F32, tag="mm", name="pmm")
                    for ck in range(2):
                        nc.tensor.matmul(
                            pmm[:], wf[ck][:, i * 128:(i + 1) * 128],
                            uT[ck][:, b * 512:(b + 1) * 512],
                            start=(ck == 0), stop=(ck == 1),
                        )
                    nc.vector.tensor_scalar_add(
                        gt[i][:, b * LW + PAD:b * LW + LW], pmm[:],
                        bias_sb[:, i:i + 1],
                    )
                for j in range(4):
                    cell0(j, b)

        # ---------------- LSTM steps 1..8 ----------------
        with ExitStack() as run:
            psg = run.enter_context(tc.tile_pool(name="psg", bufs=2, space="PSUM"))
            for k in range(1, K):
                for b in range(B_PER_CORE):
                    for j in range(4):
                        P = psg.tile([128, G4], F32, tag="P", name="P")
                        # G slice first (ready early; starts each bank's group)
                        for q in range(4):
                            nc.tensor.matmul(
                                P[:, q * 512:(q + 1) * 512], ident[:],
                                gt[4 * q + j][:, b * LW + k:b * LW + k + 512],
                                start=True, stop=False,
                            )
                        for q in range(4):
                            for dk in range(4):
                                nc.tensor.matmul(
                                    P[:, q * 512:(q + 1) * 512],
                                    whh[dk][:, (4 * q + j) * 128:(4 * q + j + 1) * 128],
                                    hT[(k + 1) % 2][dk][:, b * 512:(b + 1) * 512],
                                    start=False, stop=(dk == 3),
                                )
                        cell(j, b, P[:], k)

    nc.compile()
    return nc


def _make_in_maps(inputs):
    x = np.asarray(inputs["x"], dtype=np.float32)
    conv_w = np.asarray(inputs["conv_w"], dtype=np.float64)
    conv_b = np.asarray(inputs["conv_b"], dtype=np.float64)
    w_ih = np.asarray(inputs["w_ih"], dtype=np.float64)
    w_hh = np.asarray(inputs["w_hh"], dtype=np.float32)
    b_ih = np.asarray(inputs["b_ih"], dtype=np.float64)
    b_hh = np.asarray(inputs["b_hh"], dtype=np.float64)

    # gate permutation: torch order i,f,g,o -> i,f,o,g
    perm = np.concatenate([
        np.arange(0, D), np.arange(D, 2 * D),
        np.arange(3 * D, 4 * D), np.arange(2 * D, 3 * D),
    ])
    wf = (w_ih @ conv_w)[perm]                                  # [2048, 256]
    bias_mm = (b_ih + b_hh + w_ih @ conv_b)[perm]               # real columns
    bias_pad = (b_ih + b_hh)[perm]                              # zero-padded columns
    whh_p = w_hh[perm]

    bias_both = np.concatenate([
        bias_mm.astype(np.float32).reshape(16, 128).T,
        bias_pad.astype(np.float32).reshape(16, 128).T,
    ], axis=1)                                                  # [128, 32]
    shared = {
        "wf": np.ascontiguousarray(wf.T.astype(np.float32)),            # [256, 2048]
        "whh": np.ascontiguousarray(whh_p.T.astype(np.float32)),        # [512, 2048]
        "bias": np.ascontiguousarray(bias_both),
    }
    in_maps = []
    for c in range(N_CORES):
        m = dict(shared)
        m["x"] = np.ascontiguousarray(
            x[c * B_PER_CORE:(c + 1) * B_PER_CORE].reshape(NT, D)
        )
        in_maps.append(m)
    return in_maps


def kernel(x, conv_w, conv_b, w_ih, w_hh, b_ih, b_hh):
    if "nc" not in _cache:
        _cache["nc"] = _build()
    nc = _cache["nc"]

    in_maps = _make_in_maps(dict(
        x=x, conv_w=conv_w, conv_b=conv_b, w_ih=w_ih, w_hh=w_hh,
        b_ih=b_ih, b_hh=b_hh,
    ))

    res = bass_utils.run_bass_kernel_spmd(nc, in_maps, core_ids=list(range(N_CORES)))
    out = np.concatenate(
        [np.ascontiguousarray(r["out"].T).reshape(B_PER_CORE, L, D)
         for r in res.results], axis=0
    )
    return out


# revision 11
# speedup vs baseline: 1.1117x; 1.0252x over previous
"""LocalRNN Trainium2 kernel: GLU -> pointwise conv -> 9-step windowed LSTM.

Full inputs in, full output out. Sharding: batch across 8 cores (2 batches/core).

v2 design notes:
- All matmuls float32r (sustains ~1 cycle/row at N=512).
- Everything on-chip in transposed [feature, token] layout; the recurrence
  needs no transposes (h is produced by DVE directly in matmul-rhs layout).
- Conv (kernel_size=1) folded into W_ih on the host: G = (W_ih@conv_w) @ u
  with bias b_ih+b_hh+W_ih@conv_b.
- Gate rows permuted host-side to I,F,O,G so one sigmoid covers 1536
  contiguous PSUM columns.
- Input-side gates G computed once per token (9x reuse across overlapping
  windows); per-step G slice enters PSUM via an identity matmul in the same
  accumulation group as the W_hh matmuls.
- Output returned in transposed layout; host does the final transpose.
"""
from contextlib import ExitStack

import numpy as np

import concourse.bass as bass
import concourse.mybir as mybir
import concourse.tile as tile
from concourse import bacc, bass_utils
from concourse.masks import make_identity

F32 = mybir.dt.float32
F32R = mybir.dt.float32r
AF = mybir.ActivationFunctionType

N_CORES = 8
B_PER_CORE = 2          # batches per core
L = 512                 # sequence length
NT = B_PER_CORE * L     # tokens per core = 1024
D = 512                 # model dim
DH = 256                # GLU half dim
G4 = 4 * D              # 2048 gate rows
K = 9                   # window size
PAD = K - 1             # 8
LW = PAD + L            # 520: per-batch padded G row width

_cache = {}


def _build():
    nc = bacc.Bacc(
        trn_type="TRN2", target_bir_lowering=False, debug=False, num_devices=N_CORES
    )

    x_d = nc.dram_tensor("x", [NT, D], F32, kind="ExternalInput").ap()
    wf_d = nc.dram_tensor("wf", [DH, G4], F32, kind="ExternalInput").ap()    # (w_ih@conv_w).T permuted
    whh_d = nc.dram_tensor("whh", [D, G4], F32, kind="ExternalInput").ap()   # w_hh.T permuted
    bias_d = nc.dram_tensor("bias", [128, 32], F32, kind="ExternalInput").ap()
    out_d = nc.dram_tensor("out", [D, NT], F32, kind="ExternalOutput").ap()  # transposed out

    with tile.TileContext(nc) as tc, ExitStack() as top:
        const_pool = top.enter_context(tc.tile_pool(name="const", bufs=1))
        w_pool = top.enter_context(tc.tile_pool(name="weights", bufs=1))
        state_pool = top.enter_context(tc.tile_pool(name="state", bufs=1))

        ident_f32 = const_pool.tile([128, 128], F32, tag="idf")
        make_identity(nc, ident_f32[:])
        ident = const_pool.tile([128, 128], F32R, tag="idr")
        nc.scalar.copy(ident[:], ident_f32[:])
        zeros8 = const_pool.tile([128, PAD], F32, tag="z8")
        nc.gpsimd.memset(zeros8[:], 0.0)
        bias_sb = const_pool.tile([128, 32], F32, tag="bias")
        nc.sync.dma_start(bias_sb[:], bias_d)

        whh = [w_pool.tile([128, G4], F32R, tag=f"whh{dk}", name=f"whh{dk}")
               for dk in range(4)]

        gt = [state_pool.tile([128, B_PER_CORE * LW], F32R, tag=f"gt{i}", name=f"gt{i}")
              for i in range(16)]
        hT = [[state_pool.tile([128, NT], F32R, tag=f"h{p}_{j}", name=f"h{p}_{j}")
               for j in range(4)] for p in range(2)]
        cT = [state_pool.tile([128, NT], F32, tag=f"c{j}", name=f"c{j}") for j in range(4)]

        tp = top.enter_context(tc.tile_pool(name="tmp", bufs=2))

        def cell0(j, b):
            """step 0: c = sig(I)*tanh(G); h = sig(O)*tanh(c)."""
            cs = cT[j][:, b * 512:(b + 1) * 512]
            hs = hT[0][j][:, b * 512:(b + 1) * 512]
            g0 = b * LW  # step-0 slice offset (pad col 0..7 + G cols 0..503)
            tI = tp.tile([128, 512], F32, tag="tG", name="tI0")
            nc.scalar.activation(tI[:], gt[0 * 4 + j][:, g0:g0 + 512], AF.Sigmoid)
            tG = tp.tile([128, 512], F32, tag="tG", name="tG0")
            nc.scalar.activation(tG[:], gt[3 * 4 + j][:, g0:g0 + 512], AF.Tanh)
            tO = tp.tile([128, 512], F32, tag="tG", name="tO0")
            nc.scalar.activation(tO[:], gt[2 * 4 + j][:, g0:g0 + 512], AF.Sigmoid)
            nc.vector.tensor_mul(cs, tI[:], tG[:])
            tTc = tp.tile([128, 512], F32, tag="tTc", name="tTc0")
            nc.scalar.activation(tTc[:], cs, AF.Tanh)
            nc.vector.tensor_mul(hs, tO[:], tTc[:])

        def cell(j, b, P, k):
            """steps 1..8: full LSTM cell from psum P [128, 2048] = I|F|O|G."""
            cs = cT[j][:, b * 512:(b + 1) * 512]
            hs = hT[k % 2][j][:, b * 512:(b + 1) * 512]
            tSig = tp.tile([128, 1536], F32, tag="tSig", name="tSig")
            nc.scalar.activation(tSig[:], P[:, 0:1536], AF.Sigmoid)
            tG = tp.tile([128, 512], F32, tag="tG", name="tG")
            nc.scalar.activation(tG[:], P[:, 1536:2048], AF.Tanh)
            t1 = tp.tile([128, 512], F32, tag="t1", name="t1")
            nc.vector.tensor_mul(t1[:], tSig[:, 0:512], tG[:])
            t2 = tp.tile([128, 512], F32, tag="tG", name="t2")
            nc.vector.tensor_mul(t2[:], tSig[:, 512:1024], cs)
            nc.vector.tensor_add(cs, t1[:], t2[:])
            tTc = tp.tile([128, 512], F32, tag="tTc", name="tTc")
            nc.scalar.activation(tTc[:], cs, AF.Tanh)
            nc.vector.tensor_mul(hs, tSig[:, 1024:1536], tTc[:])
            if k == K - 1:
                nc.sync.dma_start(
                    out_d[j * 128:(j + 1) * 128, b * 512:(b + 1) * 512].bitcast(F32R),
                    hs,
                )

        # one uniform PSUM pool for the whole kernel: 2 slots x 4 banks
        psg = top.enter_context(tc.tile_pool(name="psg", bufs=2, space="PSUM"))

        # ---------------- prep: GLU -> u -> G table; step 0 interleaved ----------------
        with ExitStack() as prep:
            xp = prep.enter_context(tc.tile_pool(name="xp", bufs=2))
            up = prep.enter_context(tc.tile_pool(name="up", bufs=2))
            utp = prep.enter_context(tc.tile_pool(name="utp", bufs=1))
            wfp = prep.enter_context(tc.tile_pool(name="wfp", bufs=1))

            # x first on the DMA queue: GLU is the critical path at kernel start
            uT = [utp.tile([128, NT], F32R, tag=f"uT{ci}", name=f"uT{ci}")
                  for ci in range(2)]
            for ti in range(8):
                xt = xp.tile([128, D], F32, tag="x", name="xt")
                nc.sync.dma_start(xt[:], x_d[ti * 128:(ti + 1) * 128, :])
                sg = up.tile([128, DH], F32, tag="sg", name="sg")
                nc.scalar.activation(sg[:], xt[:, DH:D], AF.Sigmoid)
                ut = up.tile([128, DH], F32R, tag="u", name="ut")
                nc.vector.tensor_mul(ut[:], xt[:, 0:DH], sg[:])
                ptp = psg.tile([128, G4], F32, tag="P", name="Ptr")
                for ci in range(2):
                    ptr = ptp[:, ci * 512:ci * 512 + 128].bitcast(F32R)
                    nc.tensor.transpose(ptr, ut[:, ci * 128:(ci + 1) * 128], ident[:])
                    nc.vector.tensor_copy(uT[ci][:, ti * 128:(ti + 1) * 128], ptr)

            wf = []
            for ck in range(2):
                t = wfp.tile([128, G4], F32R, tag=f"wf{ck}", name=f"wf{ck}")
                nc.sync.dma_start(t[:], wf_d[ck * 128:(ck + 1) * 128, :].bitcast(F32R))
                wf.append(t)
            for dk in range(4):
                nc.sync.dma_start(
                    whh[dk][:], whh_d[dk * 128:(dk + 1) * 128, :].bitcast(F32R)
                )

            # G table per batch; step-0 cells interleaved so the PE always has
            # matmul work while ACT/DVE run the matmul-free step-0 cells
            for b in range(B_PER_CORE):
                for i in range(16):
                    nc.scalar.activation(
                        gt[i][:, b * LW:b * LW + PAD], zeros8[:],
                        AF.Identity, bias=bias_sb[:, 16 + i:16 + i + 1],
                    )
                for j in range(4):
                    P = psg.tile([128, G4], F32, tag="P", name="Pg")
                    for q in range(4):
                        for ck in range(2):
                            nc.tensor.matmul(
                                P[:, q * 512:(q + 1) * 512],
                                wf[ck][:, (4 * q + j) * 128:(4 * q + j + 1) * 128],
                                uT[ck][:, b * 512:(b + 1) * 512],
                                start=(ck == 0), stop=(ck == 1),
                            )
                    for q in range(4):
                        nc.vector.tensor_scalar_add(
                            gt[4 * q + j][:, b * LW + PAD:b * LW + LW],
                            P[:, q * 512:(q + 1) * 512],
                            bias_sb[:, 4 * q + j:4 * q + j + 1],
                        )
                for j in range(4):
                    cell0(j, b)

        # ---------------- LSTM steps 1..8 ----------------
        for k in range(1, K):
            for b in range(B_PER_CORE):
                for j in range(4):
                    P = psg.tile([128, G4], F32, tag="P", name="P")
                    # G slice first (ready early; starts each bank's group)
                    for q in range(4):
                        nc.tensor.matmul(
                            P[:, q * 512:(q + 1) * 512], ident[:],
                            gt[4 * q + j][:, b * LW + k:b * LW + k + 512],
                            start=True, stop=False,
                        )
                    for q in range(4):
                        for dk in range(4):
                            nc.tensor.matmul(
                                P[:, q * 512:(q + 1) * 512],
                                whh[dk][:, (4 * q + j) * 128:(4 * q + j + 1) * 128],
                                hT[(k + 1) % 2][dk][:, b * 512:(b + 1) * 512],
                                start=False, stop=(dk == 3),
                            )
                    cell(j, b, P[:], k)

    nc.compile()
    return nc


def _make_in_maps(inputs):
    x = np.asarray(inputs["x"], dtype=np.float32)
    conv_w = np.asarray(inputs["conv_w"], dtype=np.float64)
    conv_b = np.asarray(inputs["conv_b"], dtype=np.float64)
    w_ih = np.asarray(inputs["w_ih"], dtype=np.float64)
    w_hh = np.asarray(inputs["w_hh"], dtype=np.float32)
    b_ih = np.asarray(inputs["b_ih"], dtype=np.float64)
    b_hh = np.asarray(inputs["b_hh"], dtype=np.float64)

    # gate permutation: torch order i,f,g,o -> i,f,o,g
    perm = np.concatenate([
        np.arange(0, D), np.arange(D, 2 * D),
        np.arange(3 * D, 4 * D), np.arange(2 * D, 3 * D),
    ])
    wf = (w_ih @ conv_w)[perm]                                  # [2048, 256]
    bias_mm = (b_ih + b_hh + w_ih @ conv_b)[perm]               # real columns
    bias_pad = (b_ih + b_hh)[perm]                              # zero-padded columns
    whh_p = w_hh[perm]

    bias_both = np.concatenate([
        bias_mm.astype(np.float32).reshape(16, 128).T,
        bias_pad.astype(np.float32).reshape(16, 128).T,
    ], axis=1)                                                  # [128, 32]
    shared = {
        "wf": np.ascontiguousarray(wf.T.astype(np.float32)),            # [256, 2048]
        "whh": np.ascontiguousarray(whh_p.T.astype(np.float32)),        # [512, 2048]
        "bias": np.ascontiguousarray(bias_both),
    }
    in_maps = []
    for c in range(N_CORES):
        m = dict(shared)
        m["x"] = np.ascontiguousarray(
            x[c * B_PER_CORE:(c + 1) * B_PER_CORE].reshape(NT, D)
        )
        in_maps.append(m)
    return in_maps


def kernel(x, conv_w, conv_b, w_ih, w_hh, b_ih, b_hh):
    if "nc" not in _cache:
        _cache["nc"] = _build()
    nc = _cache["nc"]

    in_maps = _make_in_maps(dict(
        x=x, conv_w=conv_w, conv_b=conv_b, w_ih=w_ih, w_hh=w_hh,
        b_ih=b_ih, b_hh=b_hh,
    ))

    res = bass_utils.run_bass_kernel_spmd(nc, in_maps, core_ids=list(range(N_CORES)))
    out = np.concatenate(
        [np.ascontiguousarray(r["out"].T).reshape(B_PER_CORE, L, D)
         for r in res.results], axis=0
    )
    return out
